# revision 7
# baseline (speedup 1.0000x reference)
"""Trainium2 Bass kernel for nn_CryptoGNN (2-layer GCN + pooled heads).

Math (same collapse as the validated baseline):
  With A = normalized adjacency (incl. self loops), P = [B,N] pooling,
  u[d] = sum_{s->d} dis[s]x[s],   zhat = (u + dis*x_self)@W1 + sqrt(deg)*b1,
  h1hat = relu(zhat);  true h1 = dis*h1hat, so the pooling matrix columns
  are pre-scaled by dis and layer 2 + heads collapse to tiny host math.

Per-core device pipeline (8-way node sharding, 12544 dst nodes/core):
  The host pre-gathers the edge source features into per-bank dst-sorted
  bf16 streams, packed per chunk with an fp8 segment-start mask and the
  int16 gather indices into ONE byte region (one DMA per chunk).
  Per dst-chunk c:
    1. DMA the chunk's packed region [xs bf16 | mask fp8 | bidx i16]
    2. DVE segmented scan: state = mask*state + value (fp32 state) ->
       the value at a segment's last element IS the node's segment sum
    3. GPSIMD ap_gather at host-known end positions -> dense per-node
       per-bank sums (empty nodes read stream slot 0 which holds 0)
    4. DVE tensor_copy fp32->bf16 into dt rows 0..120; rows 121..127 of
       dt hold [dis*x_self; sqrt(deg)] (DMA'd once), so ONE matmul per
       tile against selw (bank-scattered W1 rows + W1;b1 on 121..127)
       computes z including the self-loop and bias terms
    5. per 8-tile batch: z (bf16 PE) -> relu -> h1 fp8 (Act),
       G^T += h1_t^T @ papt_t (both fp8) into one [128,80] PSUM across
       all 98 tiles; papt is host-swizzled to the device layout so its
       DMA is a contiguous full-rate transfer, and fp8 halves its bytes.
Host sums the 8 partial G^T and runs the small head in numpy.
"""

import sys

if "/opt/trn_rl_repo" not in sys.path:
    sys.path.insert(0, "/opt/trn_rl_repo")

import numpy as np
import ml_dtypes

N = 100000
E = 600000
B = 64
IN = 6
H = 128
S = 16

NG = 8                    # banks and cores
NS = 12544                # nodes per core shard (98*128)
NPAD = NS * NG            # 100352
NT = 98                   # node tiles per shard
# dst chunks per core, in node tiles; small first chunks ramp the pipeline
# up fast, small tail shrinks the last gather->cvt->z->relu->G chain
TCH = (1, 2, 4, 8, 16, 16, 16, 16, 12, 4, 2, 1)
C = len(TCH)
NDCS = tuple(t * 128 for t in TCH)
DOFF = tuple(int(x) for x in np.concatenate([[0], np.cumsum(NDCS)]))
PCOL = 80                 # papt columns: 64 PA + <=16 local P
P128 = 128
AUGR = 121                # dt rows 121..127 hold [dis*x_self(6); sqrt(deg)]

_compiled = {}


def _region_layout(JWS):
    """Per-chunk packed byte region: [xs W | mask W | bidx nd/8 | pad]."""
    RO = [0]
    for c in range(C):
        w = int(JWS[c])
        width = 2 * w + NDCS[c] // 8
        width = (width + 31) & ~31
        RO.append(RO[-1] + width)
    return RO


def _build_nc(JWS):
    import concourse.bacc as bacc
    import concourse.mybir as mybir
    from concourse import tile

    f32 = mybir.dt.float32
    bf16 = mybir.dt.bfloat16
    fp8 = mybir.dt.float8e4
    i16 = mybir.dt.int16

    RO = _region_layout(JWS)
    XMW = RO[-1]
    JWMAX = max(int(w) for w in JWS)
    NBMAX = max(NDCS)

    nc = bacc.Bacc("TRN2", target_bir_lowering=False, debug=False)

    xm = nc.declare_dram_parameter("xm", [P128, XMW], fp8, isOutput=False)
    aug = nc.declare_dram_parameter("aug", [7, NS], bf16, isOutput=False)
    selw = nc.declare_dram_parameter("selw", [P128, H], bf16, isOutput=False)
    papt = nc.declare_dram_parameter("papt", [P128, NT * PCOL], fp8, isOutput=False)
    gout = nc.declare_dram_parameter("gout", [P128, PCOL], f32, isOutput=True)

    with tile.TileContext(nc) as tc:
        with (
            tc.tile_pool(name="big", bufs=1) as big,
            tc.tile_pool(name="small", bufs=1) as small,
            tc.tile_pool(name="scp", bufs=3) as scp,
            tc.tile_pool(name="d32p", bufs=2) as d32p,
            tc.tile_pool(name="hbuf", bufs=4) as hbuf,
            tc.tile_pool(name="psz", bufs=3, space="PSUM") as pszp,
            tc.tile_pool(name="psG", bufs=1, space="PSUM") as psGp,
        ):
            # preload the activation-function table while DMAs run
            warm = small.tile([1, 2], f32)
            nc.vector.memset(warm[:], 0.0)
            nc.scalar.activation(out=warm[:], in_=warm[:],
                                 func=mybir.ActivationFunctionType.Copy)

            xm_t = big.tile([P128, XMW], fp8, tag="xmb")
            dt = big.tile([P128, NS], bf16, tag="dt")
            papt_t = big.tile([P128, NT * PCOL], fp8, tag="papt")

            selw_t = small.tile([P128, H], bf16)

            scs = [None] * C
            d32s = [None] * C

            def dma_xm(c0, c1):
                nc.sync.dma_start(out=xm_t[:, RO[c0]:RO[c1]],
                                  in_=xm[:, RO[c0]:RO[c1]])

            def dma_papt(t0, t1):
                p0, p1 = t0 * PCOL, t1 * PCOL
                nc.sync.dma_start(out=papt_t[:, p0:p1], in_=papt[:, p0:p1])

            def scan_c(c):
                o = RO[c]
                w = int(JWS[c])
                sc = scp.tile([P128, JWMAX], f32, tag=f"sc{c % 3}",
                              name=f"scan_{c}")
                nc.vector.tensor_tensor_scan(
                    out=sc[:, 0:w],
                    data0=xm_t[:, o + w:o + 2 * w],
                    data1=xm_t[:, o:o + w],
                    initial=0.0, op0=mybir.AluOpType.mult,
                    op1=mybir.AluOpType.add,
                )
                scs[c] = sc

            def g2_c(c):
                nd = NDCS[c]
                o = RO[c] + 2 * int(JWS[c])
                d32 = d32p.tile([P128, NBMAX], f32, tag=f"d32{c % 2}",
                                name=f"d32_{c}")
                nc.gpsimd.ap_gather(
                    out_ap=d32[:, 0:nd], in_ap=scs[c][:, 0:int(JWS[c])],
                    idxs_ap=xm_t[:, o:o + nd // 8].bitcast(i16),
                    channels=P128, num_elems=int(JWS[c]), d=1, num_idxs=nd,
                )
                d32s[c] = d32

            def cvt_c(c):
                d0, nd = DOFF[c], NDCS[c]
                nc.vector.tensor_copy(out=dt[0:AUGR, d0:d0 + nd],
                                      in_=d32s[c][0:AUGR, 0:nd])

            # ---------- issue order ----------
            dma_xm(0, 1)
            dma_xm(1, 5)
            nc.sync.dma_start(out=selw_t[:], in_=selw[:])
            nc.sync.dma_start(out=dt[AUGR:AUGR + 7, :], in_=aug[:])
            dma_xm(5, 6)
            dma_xm(6, 7)
            dma_papt(0, 46)
            dma_xm(7, 8)
            dma_xm(8, 9)
            dma_xm(9, C)
            dma_papt(46, NT)

            # interleaved per-chunk pipeline: DVE scans lead Pool gathers by
            # one chunk; converts trail gathers by one
            scan_c(0)
            scan_c(1)
            g2_c(0)
            for c in range(2, C):
                scan_c(c)
                g2_c(c - 1)
                cvt_c(c - 2)
            g2_c(C - 1)
            cvt_c(C - 2)
            cvt_c(C - 1)

            # ---------- phase B: z -> relu -> G (sw-pipelined batches) ----------
            G_ps = psGp.tile([P128, PCOL], f32, tag="G")
            QB = 8
            batches = []
            for c in range(C):
                t = DOFF[c] // 128
                left = TCH[c]
                while left > 0:
                    sz = min(QB, left)
                    batches.append((t, sz))
                    t += sz
                    left -= sz

            def z_mms(t0, m, ps):
                for u in range(m):
                    n0 = (t0 + u) * P128
                    nc.tensor.matmul(
                        out=ps[:, u * H:(u + 1) * H],
                        lhsT=dt[:, n0:n0 + P128], rhs=selw_t[:],
                        start=True, stop=True,
                    )

            def g_mms(t0, m, h1):
                for u in range(m):
                    t = t0 + u
                    nc.tensor.matmul(
                        out=G_ps[:],
                        lhsT=h1[:, u * H:(u + 1) * H],
                        rhs=papt_t[:, t * PCOL:(t + 1) * PCOL],
                        start=(t == 0), stop=(t == NT - 1),
                    )

            prev = None
            NBAT = len(batches)
            for bi, (t0, m) in enumerate(batches):
                ps = pszp.tile([P128, QB * H], f32, tag="z")
                z_mms(t0, m, ps)
                h1 = hbuf.tile([P128, QB * H], fp8, tag="h1")
                if bi == NBAT - 2:
                    # DVE is free once the last cvt retires; split this relu
                    # across Act and DVE so the tail batch finishes sooner
                    hm = (m // 2) * H
                    nc.scalar.activation(
                        out=h1[:, :hm], in_=ps[:, :hm],
                        func=mybir.ActivationFunctionType.Relu,
                    )
                    nc.vector.tensor_scalar_max(
                        out=h1[:, hm:m * H], in0=ps[:, hm:m * H], scalar1=0.0,
                    )
                else:
                    nc.scalar.activation(
                        out=h1[:, :m * H], in_=ps[:, :m * H],
                        func=mybir.ActivationFunctionType.Relu,
                    )
                if prev is not None:
                    g_mms(*prev)
                prev = (t0, m, h1)
            g_mms(*prev)

            G_sb = small.tile([P128, PCOL], f32)
            nc.vector.tensor_copy(out=G_sb[:], in_=G_ps[:])
            nc.sync.dma_start(out=gout[:], in_=G_sb[:])

    nc.compile()
    return nc


def _preprocess(x, edge_index, batch_idx):
    """Integer/structure preprocessing -> per-core device inputs."""
    src = np.asarray(edge_index[0], dtype=np.int64)
    dst = np.asarray(edge_index[1], dtype=np.int64)

    deg = (np.bincount(dst, minlength=N) + 1).astype(np.float32)
    dis = (1.0 / np.sqrt(deg)).astype(np.float32)
    sq = np.sqrt(deg).astype(np.float32)
    dis_pad = np.zeros(NPAD, np.float32)
    dis_pad[:N] = dis
    sq_pad = np.zeros(NPAD, np.float32)
    sq_pad[:N] = sq

    bi = np.asarray(batch_idx, dtype=np.int64)
    cnt = np.bincount(bi, minlength=B).astype(np.float32)

    x_np = np.asarray(x, dtype=np.float32)
    x_pad = np.zeros((NPAD, IN), np.float32)
    x_pad[:N] = x_np
    disx = x_pad * dis_pad[:, None]          # [NPAD, 6]

    # ---- pooling matrices (dense PA = P @ A) ----
    loop = np.arange(N, dtype=np.int64)
    src2 = np.concatenate([src, loop])
    dst2 = np.concatenate([dst, loop])
    w = (dis[src2] * dis[dst2]).astype(np.float64)
    flat = bi[dst2] * NPAD + src2
    PA = np.bincount(flat, weights=w, minlength=B * NPAD).reshape(B, NPAD)
    PA = PA.astype(np.float32)
    Pm = np.zeros((B, NPAD), np.float32)
    Pm[bi, np.arange(N)] = 1.0
    papt_full = (np.concatenate([PA, Pm], axis=0) * dis_pad[None, :]).T  # [NPAD,128]

    # graph span per core (for the P columns)
    first_graph = np.zeros(NG, np.int64)
    span = np.zeros(NG, np.int64)
    for k in range(NG):
        lo, hi = k * NS, min((k + 1) * NS, N)
        if lo >= N:
            first_graph[k] = B - 1
            span[k] = 1
            continue
        gset = bi[lo:hi]
        first_graph[k] = gset[0]
        span[k] = gset[-1] - gset[0] + 1
        assert span[k] <= PCOL - B, f"graph span {span[k]} > {PCOL - B}"

    # ---- per (core, chunk) streams, edges round-robin balanced on banks ----
    core = dst // NS
    dst_local = dst - core * NS
    chunk = np.searchsorted(np.asarray(DOFF[1:]), dst_local, side="right")
    # sort by (core, chunk, dst_local); bank = rank within group mod NG
    key0 = (core * C + chunk) * NS + dst_local
    order0 = np.argsort(key0, kind="stable")
    grp = (core * C + chunk)[order0]
    rank = np.arange(E) - np.concatenate(
        [[0], np.cumsum(np.bincount(grp, minlength=NG * C))])[grp]
    bank_e = np.empty(E, np.int64)
    bank_e[order0] = rank % NG

    # final order: (core, chunk, bank, dst_local)
    key = (((core * C + chunk) * NG + bank_e)) * NS + dst_local
    order = np.argsort(key, kind="stable")
    src_s = src[order]
    dstl_s = dst_local[order]

    cell = ((core * C + chunk) * NG + bank_e)[order]
    cellcnt = np.bincount(cell, minlength=NG * C * NG)
    cell_starts = np.zeros(NG * C * NG + 1, np.int64)
    np.cumsum(cellcnt, out=cell_starts[1:])
    cc = cellcnt.reshape(NG, C, NG)

    # per-chunk stream widths (+1 lead 0-slot, pad to 32)
    JWS = []
    for c in range(C):
        m = int(cc[:, c, :].max())
        JWS.append(((m + 1 + 31) // 32) * 32)
    RO = _region_layout(JWS)
    XMW = RO[-1]

    # packed per-chunk regions: [xs bf16 bytes | mask fp8 | bidx i16 | pad]
    f8 = ml_dtypes.float8_e4m3
    xm_all = np.zeros((NG, P128, XMW), f8)
    disx_f8 = disx.astype(f8)
    for k in range(NG):
        for c in range(C):
            w = int(JWS[c])
            nd = NDCS[c]
            b0 = RO[c]
            xs_c = np.zeros((P128, w), f8)
            mk_c = np.zeros((P128, w), f8)
            bx_c = np.zeros((P128, nd // 16), np.int16)
            for g in range(NG):
                ci = (k * C + c) * NG + g
                s0, s1 = cell_starts[ci], cell_starts[ci + 1]
                ncell = s1 - s0
                # pre-gathered feature stream (lead slot 0 stays 0.0)
                xs_c[16 * g:16 * g + 6, 1:1 + ncell] = disx_f8[src_s[s0:s1]].T
                # mask: 0 at each dst segment's first edge, 1 inside
                if ncell > 0:
                    dl = dstl_s[s0:s1]
                    m = np.ones(ncell, f8)
                    m[0] = 0.0
                    m[1:][dl[1:] != dl[:-1]] = 0.0
                    mk_c[16 * g:16 * g + 6, 1:1 + ncell] = m

                # per-node segment end positions (0 for empty -> reads the
                # 0.0 lead slot)
                dloc = dstl_s[s0:s1] - DOFF[c]
                cnts = np.bincount(dloc, minlength=nd)
                ends = np.cumsum(cnts)
                bvals = np.where(cnts > 0, ends, 0).astype(np.int64)
                bx_c[16 * g:16 * (g + 1)] = (
                    bvals.reshape(nd // 16, 16).T.astype(np.int16)
                )
            xm_all[k, :, b0:b0 + w] = xs_c
            xm_all[k, :, b0 + w:b0 + 2 * w] = mk_c
            xm_all[k, :, b0 + 2 * w:b0 + 2 * w + nd // 8] = bx_c.view(f8)

    # aug rows for dt[121:128]: 0-5 dis*x own chunk (self loop), 6 sqrt(deg)
    aug_all = np.zeros((NG, 7, NS), ml_dtypes.bfloat16)
    for k in range(NG):
        n0 = k * NS
        aug_all[k, 0:6] = disx[n0:n0 + NS].T.astype(ml_dtypes.bfloat16)
        aug_all[k, 6] = sq_pad[n0:n0 + NS].astype(ml_dtypes.bfloat16)

    # papt per core: 64 PA cols + local P cols, swizzled to the device
    # tile layout [128, NT*PCOL] so the DMA is a contiguous transfer
    papt_all = np.zeros((NG, P128, NT * PCOL), f8)
    for k in range(NG):
        n0 = k * NS
        pk = np.zeros((NS, PCOL), np.float32)
        pk[:, :B] = papt_full[n0:n0 + NS, :B]
        b0, sp = first_graph[k], span[k]
        pk[:, B:B + sp] = papt_full[n0:n0 + NS, B + b0:B + b0 + sp]
        papt_all[k] = (
            pk.reshape(NT, P128, PCOL).transpose(1, 0, 2)
            .reshape(P128, NT * PCOL).astype(f8)
        )

    return {
        "JW": tuple(JWS),
        "JWS": JWS,
        "xm_all": xm_all,
        "aug_all": aug_all,
        "papt_all": papt_all,
        "first_graph": first_graph,
        "span": span,
        "cnt": cnt,
    }


def _head(G, cnt, inputs):
    f = np.float32
    W2 = np.asarray(inputs["W2"], f)
    b2 = np.asarray(inputs["b2"], f)
    Wg = np.asarray(inputs["Wg"], f)
    bg = np.asarray(inputs["bg"], f)
    Et = np.asarray(inputs["Et"], f)
    Ek = np.asarray(inputs["Ek"], f)
    Ev = np.asarray(inputs["Ev"], f)
    Wp = np.asarray(inputs["Wp"], f)
    bp = np.asarray(inputs["bp"], f)
    Ekid = np.asarray(inputs["Ekid"], f)
    Wc = np.asarray(inputs["Wc"], f)
    bc = np.asarray(inputs["bc"], f)
    Wl = np.asarray(inputs["Wl"], f)
    bl = np.asarray(inputs["bl"], f)
    Wm1 = np.asarray(inputs["Wm1"], f)
    bm1 = np.asarray(inputs["bm1"], f)
    Wm2 = np.asarray(inputs["Wm2"], f)
    bm2 = np.asarray(inputs["bm2"], f)
    st = np.asarray(inputs["sol_type_idx"], np.int64)
    sk = np.asarray(inputs["sol_key_idx"], np.int64)
    sv = np.asarray(inputs["sol_val_idx"], np.int64)
    kid = np.asarray(inputs["kernel_id"], np.int64)
    cond = np.asarray(inputs["cond_vec"], f)
    loc = np.asarray(inputs["local_feats"], f)

    relu = lambda a: np.maximum(a, 0.0).astype(f)

    Ph2 = G[:B] @ W2 + cnt[:, None] * b2[None, :] + G[B:]
    g = (Ph2 / np.maximum(cnt, 1.0)[:, None]) @ Wg + bg

    seq_mean = np.concatenate(
        [Et[st].mean(axis=1), Ek[sk].mean(axis=1), Ev[sv].mean(axis=1)], axis=-1
    ).astype(f)
    p = relu(seq_mean @ Wp + bp)
    kvec = Ekid[kid]
    c = relu(cond @ Wc + bc)
    l = relu(loc @ Wl + bl)
    xf = np.concatenate([g, p, kvec, c, l], axis=1).astype(f)
    return (relu(xf @ Wm1 + bm1) @ Wm2 + bm2).astype(f)


def kernel(**inputs) -> np.ndarray:
    from concourse.bass_utils import run_bass_kernel_spmd

    pre = _preprocess(inputs["x"], inputs["edge_index"], inputs["batch_idx"])
    sig = pre["JW"]
    if sig not in _compiled:
        _compiled[sig] = _build_nc(tuple(pre["JWS"]))
    nc = _compiled[sig]

    W1 = np.asarray(inputs["W1"], np.float32)
    b1 = np.asarray(inputs["b1"], np.float32)
    selw = np.zeros((P128, H), ml_dtypes.bfloat16)
    for g in range(NG):
        selw[16 * g:16 * g + 6] = W1.astype(ml_dtypes.bfloat16)
    selw[AUGR:AUGR + 6] = W1.astype(ml_dtypes.bfloat16)
    selw[AUGR + 6] = b1.astype(ml_dtypes.bfloat16)

    in_maps = []
    for k in range(NG):
        in_maps.append({
            "xm": pre["xm_all"][k],
            "aug": pre["aug_all"][k],
            "selw": selw,
            "papt": pre["papt_all"][k],
        })

    res = run_bass_kernel_spmd(nc, in_maps, core_ids=list(range(NG)))

    Gpa = np.zeros((B, H), np.float64)
    Gp = np.zeros((B, H), np.float64)
    for k, r in enumerate(res.results):
        gt = r["gout"].astype(np.float64)      # [128 f, 80 c]
        Gpa += gt[:, :B].T
        b0, sp = pre["first_graph"][k], pre["span"][k]
        Gp[b0:b0 + sp] += gt[:, B:B + sp].T
    G = np.concatenate([Gpa, Gp], axis=0).astype(np.float32)   # [128, H]

    return _head(G, pre["cnt"], inputs)


# revision 8
# speedup vs baseline: 1.1089x; 1.1089x over previous
"""Trainium2 Bass kernel for nn_CryptoGNN (2-layer GCN + pooled heads).

Math (same collapse as the validated baseline):
  With A = normalized adjacency (incl. self loops), P = [B,N] pooling,
  u[d] = sum_{s->d} dis[s]x[s],   zhat = (u + dis*x_self)@W1 + sqrt(deg)*b1,
  h1hat = relu(zhat);  true h1 = dis*h1hat, so the pooling matrix columns
  are pre-scaled by dis and layer 2 + heads collapse to tiny host math.

Per-core device pipeline (8-way node sharding, 12544 dst nodes/core):
  The host pre-gathers the edge source features into per-bank dst-sorted
  bf16 streams, packed per chunk with an fp8 segment-start mask and the
  int16 gather indices into ONE byte region (one DMA per chunk).
  Per dst-chunk c:
    1. DMA the chunk's packed region [xs bf16 | mask fp8 | bidx i16]
    2. DVE segmented scan: state = mask*state + value (fp32 state) ->
       the value at a segment's last element IS the node's segment sum
    3. GPSIMD ap_gather at host-known end positions -> dense per-node
       per-bank sums (empty nodes read stream slot 0 which holds 0)
    4. DVE tensor_copy fp32->bf16 into dt rows 0..120; rows 121..127 of
       dt hold [dis*x_self; sqrt(deg)] (DMA'd once), so ONE matmul per
       tile against selw (bank-scattered W1 rows + W1;b1 on 121..127)
       computes z including the self-loop and bias terms
    5. per 8-tile batch: z (bf16 PE) -> relu -> h1 fp8 (Act),
       G^T += h1_t^T @ papt_t (both fp8) into one [128,80] PSUM across
       all 98 tiles; papt is host-swizzled to the device layout so its
       DMA is a contiguous full-rate transfer, and fp8 halves its bytes.
Host sums the 8 partial G^T and runs the small head in numpy.
"""

import sys

if "/opt/trn_rl_repo" not in sys.path:
    sys.path.insert(0, "/opt/trn_rl_repo")

import numpy as np
import ml_dtypes

N = 100000
E = 600000
B = 64
IN = 6
H = 128
S = 16

NG = 8                    # banks and cores
NS = 12544                # nodes per core shard (98*128)
NPAD = NS * NG            # 100352
NT = 98                   # node tiles per shard
# dst chunks per core, in node tiles; small first chunks ramp the pipeline
# up fast, small tail shrinks the last gather->cvt->z->relu->G chain
TCH = (1, 2, 4, 8, 16, 16, 16, 16, 8, 6, 3, 2)
C = len(TCH)
NDCS = tuple(t * 128 for t in TCH)
DOFF = tuple(int(x) for x in np.concatenate([[0], np.cumsum(NDCS)]))
PCOL = 80                 # papt columns: 64 PA + <=16 local P
P128 = 128
AUGR = 121                # dt rows 121..127 hold [dis*x_self(6); sqrt(deg)]

_compiled = {}


def _region_layout(JWS):
    """Per-chunk packed byte region: [xs W | mask W | bidx nd/8 | pad]."""
    RO = [0]
    for c in range(C):
        w = int(JWS[c])
        width = 2 * w + NDCS[c] // 8
        width = (width + 31) & ~31
        RO.append(RO[-1] + width)
    return RO


def _build_nc(JWS):
    import concourse.bacc as bacc
    import concourse.mybir as mybir
    from concourse import tile

    f32 = mybir.dt.float32
    bf16 = mybir.dt.bfloat16
    fp8 = mybir.dt.float8e4
    i16 = mybir.dt.int16

    RO = _region_layout(JWS)
    XMW = RO[-1]
    JWMAX = max(int(w) for w in JWS)
    NBMAX = max(NDCS)

    nc = bacc.Bacc("TRN2", target_bir_lowering=False, debug=False)

    xm = nc.declare_dram_parameter("xm", [P128, XMW], fp8, isOutput=False)
    aug = nc.declare_dram_parameter("aug", [7, NS], bf16, isOutput=False)
    selw = nc.declare_dram_parameter("selw", [P128, H], bf16, isOutput=False)
    papt = nc.declare_dram_parameter("papt", [P128, NT * PCOL], fp8, isOutput=False)
    gout = nc.declare_dram_parameter("gout", [P128, PCOL], f32, isOutput=True)

    with tile.TileContext(nc) as tc:
        with (
            tc.tile_pool(name="big", bufs=1) as big,
            tc.tile_pool(name="small", bufs=1) as small,
            tc.tile_pool(name="scp", bufs=3) as scp,
            tc.tile_pool(name="d32p", bufs=2) as d32p,
            tc.tile_pool(name="hbuf", bufs=6) as hbuf,
            tc.tile_pool(name="psz", bufs=3, space="PSUM") as pszp,
            tc.tile_pool(name="psG", bufs=1, space="PSUM") as psGp,
        ):
            # preload the activation-function table while DMAs run
            warm = small.tile([1, 2], f32)
            nc.vector.memset(warm[:], 0.0)
            nc.scalar.activation(out=warm[:], in_=warm[:],
                                 func=mybir.ActivationFunctionType.Copy)

            xm_t = big.tile([P128, XMW], fp8, tag="xmb")
            dt = big.tile([P128, NS], bf16, tag="dt")
            papt_t = big.tile([P128, NT * PCOL], fp8, tag="papt")

            selw_t = small.tile([P128, H], bf16)

            scs = [None] * C
            d32s = [None] * C

            def dma_xm(c0, c1):
                nc.sync.dma_start(out=xm_t[:, RO[c0]:RO[c1]],
                                  in_=xm[:, RO[c0]:RO[c1]])

            def dma_papt(t0, t1):
                p0, p1 = t0 * PCOL, t1 * PCOL
                nc.sync.dma_start(out=papt_t[:, p0:p1], in_=papt[:, p0:p1])

            def scan_c(c):
                o = RO[c]
                w = int(JWS[c])
                sc = scp.tile([P128, JWMAX], f32, tag=f"sc{c % 3}",
                              name=f"scan_{c}")
                nc.vector.tensor_tensor_scan(
                    out=sc[:, 0:w],
                    data0=xm_t[:, o + w:o + 2 * w],
                    data1=xm_t[:, o:o + w],
                    initial=0.0, op0=mybir.AluOpType.mult,
                    op1=mybir.AluOpType.add,
                )
                scs[c] = sc

            def g2_c(c):
                nd = NDCS[c]
                o = RO[c] + 2 * int(JWS[c])
                d32 = d32p.tile([P128, NBMAX], f32, tag=f"d32{c % 2}",
                                name=f"d32_{c}")
                nc.gpsimd.ap_gather(
                    out_ap=d32[:, 0:nd], in_ap=scs[c][:, 0:int(JWS[c])],
                    idxs_ap=xm_t[:, o:o + nd // 8].bitcast(i16),
                    channels=P128, num_elems=int(JWS[c]), d=1, num_idxs=nd,
                )
                d32s[c] = d32

            def cvt_c(c):
                d0, nd = DOFF[c], NDCS[c]
                nc.vector.tensor_copy(out=dt[0:AUGR, d0:d0 + nd],
                                      in_=d32s[c][0:AUGR, 0:nd])

            # ---------- issue order ----------
            dma_xm(0, 1)
            dma_xm(1, 2)
            dma_xm(2, 3)
            nc.sync.dma_start(out=selw_t[:], in_=selw[:])
            nc.sync.dma_start(out=dt[AUGR:AUGR + 7, :], in_=aug[:])
            dma_xm(3, 4)
            dma_xm(4, 5)
            dma_xm(5, 6)
            dma_xm(6, 7)
            dma_papt(0, 46)
            dma_xm(7, 8)
            dma_xm(8, 9)
            dma_papt(46, NT)
            dma_xm(9, C)

            # interleaved per-chunk pipeline: DVE scans lead Pool gathers by
            # one chunk; converts trail gathers by one
            scan_c(0)
            scan_c(1)
            g2_c(0)
            for c in range(2, C):
                scan_c(c)
                g2_c(c - 1)
                cvt_c(c - 2)
            g2_c(C - 1)
            cvt_c(C - 2)
            cvt_c(C - 1)

            # ---------- phase B: z -> relu -> G (sw-pipelined batches) ----------
            G_ps = psGp.tile([P128, PCOL], f32, tag="G")
            QB = 8
            batches = []
            for c in range(C):
                t = DOFF[c] // 128
                left = TCH[c]
                while left > 0:
                    sz = min(QB, left)
                    batches.append((t, sz))
                    t += sz
                    left -= sz

            def z_mms(t0, m, ps):
                for u in range(m):
                    n0 = (t0 + u) * P128
                    nc.tensor.matmul(
                        out=ps[:, u * H:(u + 1) * H],
                        lhsT=dt[:, n0:n0 + P128], rhs=selw_t[:],
                        start=True, stop=True,
                    )

            def g_mms(t0, m, h1):
                for u in range(m):
                    t = t0 + u
                    nc.tensor.matmul(
                        out=G_ps[:],
                        lhsT=h1[:, u * H:(u + 1) * H],
                        rhs=papt_t[:, t * PCOL:(t + 1) * PCOL],
                        start=(t == 0), stop=(t == NT - 1),
                    )

            prev = None
            NBAT = len(batches)
            for bi, (t0, m) in enumerate(batches):
                ps = pszp.tile([P128, QB * H], f32, tag="z")
                z_mms(t0, m, ps)
                h1 = hbuf.tile([P128, QB * H], fp8, tag="h1")
                if t0 >= DOFF[8]:
                    # DVE drains its cvts before Act drains relus in the
                    # tail; split late relus across Act and DVE
                    hm = ((m + 1) // 2) * H
                    nc.scalar.activation(
                        out=h1[:, :hm], in_=ps[:, :hm],
                        func=mybir.ActivationFunctionType.Relu,
                    )
                    if hm < m * H:
                        nc.vector.tensor_scalar_max(
                            out=h1[:, hm:m * H], in0=ps[:, hm:m * H],
                            scalar1=0.0,
                        )
                else:
                    nc.scalar.activation(
                        out=h1[:, :m * H], in_=ps[:, :m * H],
                        func=mybir.ActivationFunctionType.Relu,
                    )
                if prev is not None:
                    g_mms(*prev)
                prev = (t0, m, h1)
            g_mms(*prev)

            G_sb = small.tile([P128, PCOL], f32)
            nc.vector.tensor_copy(out=G_sb[:], in_=G_ps[:])
            nc.sync.dma_start(out=gout[:], in_=G_sb[:])

    nc.compile()
    return nc


def _preprocess(x, edge_index, batch_idx):
    """Integer/structure preprocessing -> per-core device inputs."""
    src = np.asarray(edge_index[0], dtype=np.int64)
    dst = np.asarray(edge_index[1], dtype=np.int64)

    deg = (np.bincount(dst, minlength=N) + 1).astype(np.float32)
    dis = (1.0 / np.sqrt(deg)).astype(np.float32)
    sq = np.sqrt(deg).astype(np.float32)
    dis_pad = np.zeros(NPAD, np.float32)
    dis_pad[:N] = dis
    sq_pad = np.zeros(NPAD, np.float32)
    sq_pad[:N] = sq

    bi = np.asarray(batch_idx, dtype=np.int64)
    cnt = np.bincount(bi, minlength=B).astype(np.float32)

    x_np = np.asarray(x, dtype=np.float32)
    x_pad = np.zeros((NPAD, IN), np.float32)
    x_pad[:N] = x_np
    disx = x_pad * dis_pad[:, None]          # [NPAD, 6]

    # ---- pooling matrices (dense PA = P @ A) ----
    loop = np.arange(N, dtype=np.int64)
    src2 = np.concatenate([src, loop])
    dst2 = np.concatenate([dst, loop])
    w = (dis[src2] * dis[dst2]).astype(np.float64)
    flat = bi[dst2] * NPAD + src2
    PA = np.bincount(flat, weights=w, minlength=B * NPAD).reshape(B, NPAD)
    PA = PA.astype(np.float32)
    Pm = np.zeros((B, NPAD), np.float32)
    Pm[bi, np.arange(N)] = 1.0
    papt_full = (np.concatenate([PA, Pm], axis=0) * dis_pad[None, :]).T  # [NPAD,128]

    # graph span per core (for the P columns)
    first_graph = np.zeros(NG, np.int64)
    span = np.zeros(NG, np.int64)
    for k in range(NG):
        lo, hi = k * NS, min((k + 1) * NS, N)
        if lo >= N:
            first_graph[k] = B - 1
            span[k] = 1
            continue
        gset = bi[lo:hi]
        first_graph[k] = gset[0]
        span[k] = gset[-1] - gset[0] + 1
        assert span[k] <= PCOL - B, f"graph span {span[k]} > {PCOL - B}"

    # ---- per (core, chunk) streams, edges round-robin balanced on banks ----
    core = dst // NS
    dst_local = dst - core * NS
    chunk = np.searchsorted(np.asarray(DOFF[1:]), dst_local, side="right")
    # sort by (core, chunk, dst_local); bank = rank within group mod NG
    key0 = (core * C + chunk) * NS + dst_local
    order0 = np.argsort(key0, kind="stable")
    grp = (core * C + chunk)[order0]
    rank = np.arange(E) - np.concatenate(
        [[0], np.cumsum(np.bincount(grp, minlength=NG * C))])[grp]
    bank_e = np.empty(E, np.int64)
    bank_e[order0] = rank % NG

    # final order: (core, chunk, bank, dst_local)
    key = (((core * C + chunk) * NG + bank_e)) * NS + dst_local
    order = np.argsort(key, kind="stable")
    src_s = src[order]
    dstl_s = dst_local[order]

    cell = ((core * C + chunk) * NG + bank_e)[order]
    cellcnt = np.bincount(cell, minlength=NG * C * NG)
    cell_starts = np.zeros(NG * C * NG + 1, np.int64)
    np.cumsum(cellcnt, out=cell_starts[1:])
    cc = cellcnt.reshape(NG, C, NG)

    # per-chunk stream widths (+1 lead 0-slot, pad to 32)
    JWS = []
    for c in range(C):
        m = int(cc[:, c, :].max())
        JWS.append(((m + 1 + 31) // 32) * 32)
    RO = _region_layout(JWS)
    XMW = RO[-1]

    # packed per-chunk regions: [xs bf16 bytes | mask fp8 | bidx i16 | pad]
    f8 = ml_dtypes.float8_e4m3
    xm_all = np.zeros((NG, P128, XMW), f8)
    disx_f8 = disx.astype(f8)
    for k in range(NG):
        for c in range(C):
            w = int(JWS[c])
            nd = NDCS[c]
            b0 = RO[c]
            xs_c = np.zeros((P128, w), f8)
            mk_c = np.zeros((P128, w), f8)
            bx_c = np.zeros((P128, nd // 16), np.int16)
            for g in range(NG):
                ci = (k * C + c) * NG + g
                s0, s1 = cell_starts[ci], cell_starts[ci + 1]
                ncell = s1 - s0
                # pre-gathered feature stream (lead slot 0 stays 0.0)
                xs_c[16 * g:16 * g + 6, 1:1 + ncell] = disx_f8[src_s[s0:s1]].T
                # mask: 0 at each dst segment's first edge, 1 inside
                if ncell > 0:
                    dl = dstl_s[s0:s1]
                    m = np.ones(ncell, f8)
                    m[0] = 0.0
                    m[1:][dl[1:] != dl[:-1]] = 0.0
                    mk_c[16 * g:16 * g + 6, 1:1 + ncell] = m

                # per-node segment end positions (0 for empty -> reads the
                # 0.0 lead slot)
                dloc = dstl_s[s0:s1] - DOFF[c]
                cnts = np.bincount(dloc, minlength=nd)
                ends = np.cumsum(cnts)
                bvals = np.where(cnts > 0, ends, 0).astype(np.int64)
                bx_c[16 * g:16 * (g + 1)] = (
                    bvals.reshape(nd // 16, 16).T.astype(np.int16)
                )
            xm_all[k, :, b0:b0 + w] = xs_c
            xm_all[k, :, b0 + w:b0 + 2 * w] = mk_c
            xm_all[k, :, b0 + 2 * w:b0 + 2 * w + nd // 8] = bx_c.view(f8)

    # aug rows for dt[121:128]: 0-5 dis*x own chunk (self loop), 6 sqrt(deg)
    aug_all = np.zeros((NG, 7, NS), ml_dtypes.bfloat16)
    for k in range(NG):
        n0 = k * NS
        aug_all[k, 0:6] = disx[n0:n0 + NS].T.astype(ml_dtypes.bfloat16)
        aug_all[k, 6] = sq_pad[n0:n0 + NS].astype(ml_dtypes.bfloat16)

    # papt per core: 64 PA cols + local P cols, swizzled to the device
    # tile layout [128, NT*PCOL] so the DMA is a contiguous transfer
    papt_all = np.zeros((NG, P128, NT * PCOL), f8)
    for k in range(NG):
        n0 = k * NS
        pk = np.zeros((NS, PCOL), np.float32)
        pk[:, :B] = papt_full[n0:n0 + NS, :B]
        b0, sp = first_graph[k], span[k]
        pk[:, B:B + sp] = papt_full[n0:n0 + NS, B + b0:B + b0 + sp]
        papt_all[k] = (
            pk.reshape(NT, P128, PCOL).transpose(1, 0, 2)
            .reshape(P128, NT * PCOL).astype(f8)
        )

    return {
        "JW": tuple(JWS),
        "JWS": JWS,
        "xm_all": xm_all,
        "aug_all": aug_all,
        "papt_all": papt_all,
        "first_graph": first_graph,
        "span": span,
        "cnt": cnt,
    }


def _head(G, cnt, inputs):
    f = np.float32
    W2 = np.asarray(inputs["W2"], f)
    b2 = np.asarray(inputs["b2"], f)
    Wg = np.asarray(inputs["Wg"], f)
    bg = np.asarray(inputs["bg"], f)
    Et = np.asarray(inputs["Et"], f)
    Ek = np.asarray(inputs["Ek"], f)
    Ev = np.asarray(inputs["Ev"], f)
    Wp = np.asarray(inputs["Wp"], f)
    bp = np.asarray(inputs["bp"], f)
    Ekid = np.asarray(inputs["Ekid"], f)
    Wc = np.asarray(inputs["Wc"], f)
    bc = np.asarray(inputs["bc"], f)
    Wl = np.asarray(inputs["Wl"], f)
    bl = np.asarray(inputs["bl"], f)
    Wm1 = np.asarray(inputs["Wm1"], f)
    bm1 = np.asarray(inputs["bm1"], f)
    Wm2 = np.asarray(inputs["Wm2"], f)
    bm2 = np.asarray(inputs["bm2"], f)
    st = np.asarray(inputs["sol_type_idx"], np.int64)
    sk = np.asarray(inputs["sol_key_idx"], np.int64)
    sv = np.asarray(inputs["sol_val_idx"], np.int64)
    kid = np.asarray(inputs["kernel_id"], np.int64)
    cond = np.asarray(inputs["cond_vec"], f)
    loc = np.asarray(inputs["local_feats"], f)

    relu = lambda a: np.maximum(a, 0.0).astype(f)

    Ph2 = G[:B] @ W2 + cnt[:, None] * b2[None, :] + G[B:]
    g = (Ph2 / np.maximum(cnt, 1.0)[:, None]) @ Wg + bg

    seq_mean = np.concatenate(
        [Et[st].mean(axis=1), Ek[sk].mean(axis=1), Ev[sv].mean(axis=1)], axis=-1
    ).astype(f)
    p = relu(seq_mean @ Wp + bp)
    kvec = Ekid[kid]
    c = relu(cond @ Wc + bc)
    l = relu(loc @ Wl + bl)
    xf = np.concatenate([g, p, kvec, c, l], axis=1).astype(f)
    return (relu(xf @ Wm1 + bm1) @ Wm2 + bm2).astype(f)


def kernel(**inputs) -> np.ndarray:
    from concourse.bass_utils import run_bass_kernel_spmd

    pre = _preprocess(inputs["x"], inputs["edge_index"], inputs["batch_idx"])
    sig = pre["JW"]
    if sig not in _compiled:
        _compiled[sig] = _build_nc(tuple(pre["JWS"]))
    nc = _compiled[sig]

    W1 = np.asarray(inputs["W1"], np.float32)
    b1 = np.asarray(inputs["b1"], np.float32)
    selw = np.zeros((P128, H), ml_dtypes.bfloat16)
    for g in range(NG):
        selw[16 * g:16 * g + 6] = W1.astype(ml_dtypes.bfloat16)
    selw[AUGR:AUGR + 6] = W1.astype(ml_dtypes.bfloat16)
    selw[AUGR + 6] = b1.astype(ml_dtypes.bfloat16)

    in_maps = []
    for k in range(NG):
        in_maps.append({
            "xm": pre["xm_all"][k],
            "aug": pre["aug_all"][k],
            "selw": selw,
            "papt": pre["papt_all"][k],
        })

    res = run_bass_kernel_spmd(nc, in_maps, core_ids=list(range(NG)))

    Gpa = np.zeros((B, H), np.float64)
    Gp = np.zeros((B, H), np.float64)
    for k, r in enumerate(res.results):
        gt = r["gout"].astype(np.float64)      # [128 f, 80 c]
        Gpa += gt[:, :B].T
        b0, sp = pre["first_graph"][k], pre["span"][k]
        Gp[b0:b0 + sp] += gt[:, B:B + sp].T
    G = np.concatenate([Gpa, Gp], axis=0).astype(np.float32)   # [128, H]

    return _head(G, pre["cnt"], inputs)


# revision 9
# speedup vs baseline: 1.1187x; 1.0088x over previous
"""Trainium2 Bass kernel for nn_CryptoGNN (2-layer GCN + pooled heads).

Math (same collapse as the validated baseline):
  With A = normalized adjacency (incl. self loops), P = [B,N] pooling,
  u[d] = sum_{s->d} dis[s]x[s],   zhat = (u + dis*x_self)@W1 + sqrt(deg)*b1,
  h1hat = relu(zhat);  true h1 = dis*h1hat, so the pooling matrix columns
  are pre-scaled by dis and layer 2 + heads collapse to tiny host math.

Per-core device pipeline (8-way node sharding, 12544 dst nodes/core):
  The host pre-gathers the edge source features into per-bank dst-sorted
  bf16 streams, packed per chunk with an fp8 segment-start mask and the
  int16 gather indices into ONE byte region (one DMA per chunk).
  Per dst-chunk c:
    1. DMA the chunk's packed region [xs bf16 | mask fp8 | bidx i16]
    2. DVE segmented scan: state = mask*state + value (fp32 state) ->
       the value at a segment's last element IS the node's segment sum
    3. GPSIMD ap_gather at host-known end positions -> dense per-node
       per-bank sums (empty nodes read stream slot 0 which holds 0)
    4. DVE tensor_copy fp32->bf16 into dt rows 0..120; rows 121..127 of
       dt hold [dis*x_self; sqrt(deg)] (DMA'd once), so ONE matmul per
       tile against selw (bank-scattered W1 rows + W1;b1 on 121..127)
       computes z including the self-loop and bias terms
    5. per 8-tile batch: z (bf16 PE) -> relu -> h1 fp8 (Act),
       G^T += h1_t^T @ papt_t (both fp8) into one [128,80] PSUM across
       all 98 tiles; papt is host-swizzled to the device layout so its
       DMA is a contiguous full-rate transfer, and fp8 halves its bytes.
Host sums the 8 partial G^T and runs the small head in numpy.
"""

import sys

if "/opt/trn_rl_repo" not in sys.path:
    sys.path.insert(0, "/opt/trn_rl_repo")

import numpy as np
import ml_dtypes

N = 100000
E = 600000
B = 64
IN = 6
H = 128
S = 16

NG = 8                    # banks and cores
NS = 12544                # nodes per core shard (98*128)
NPAD = NS * NG            # 100352
NT = 98                   # node tiles per shard
# dst chunks per core, in node tiles; small first chunks ramp the pipeline
# up fast, small tail shrinks the last gather->cvt->z->relu->G chain
TCH = (1, 2, 4, 8, 12, 16, 16, 16, 12, 6, 3, 2)
C = len(TCH)
NDCS = tuple(t * 128 for t in TCH)
DOFF = tuple(int(x) for x in np.concatenate([[0], np.cumsum(NDCS)]))
PCOL = 80                 # papt columns: 64 PA + <=16 local P
P128 = 128
AUGR = 121                # dt rows 121..127 hold [dis*x_self(6); sqrt(deg)]

_compiled = {}


def _region_layout(JWS):
    """Per-chunk packed byte region: [xs W | mask W | bidx nd/8 | pad]."""
    RO = [0]
    for c in range(C):
        w = int(JWS[c])
        width = 2 * w + NDCS[c] // 8
        width = (width + 31) & ~31
        RO.append(RO[-1] + width)
    return RO


def _build_nc(JWS):
    import concourse.bacc as bacc
    import concourse.mybir as mybir
    from concourse import tile

    f32 = mybir.dt.float32
    bf16 = mybir.dt.bfloat16
    fp8 = mybir.dt.float8e4
    i16 = mybir.dt.int16

    RO = _region_layout(JWS)
    XMW = RO[-1]
    JWMAX = max(int(w) for w in JWS)
    NBMAX = max(NDCS)

    nc = bacc.Bacc("TRN2", target_bir_lowering=False, debug=False)

    xm = nc.declare_dram_parameter("xm", [P128, XMW], fp8, isOutput=False)
    aug = nc.declare_dram_parameter("aug", [7, NS], bf16, isOutput=False)
    selw = nc.declare_dram_parameter("selw", [P128, H], bf16, isOutput=False)
    papt = nc.declare_dram_parameter("papt", [P128, NT * PCOL], fp8, isOutput=False)
    gout = nc.declare_dram_parameter("gout", [P128, PCOL], f32, isOutput=True)

    with tile.TileContext(nc) as tc:
        with (
            tc.tile_pool(name="big", bufs=1) as big,
            tc.tile_pool(name="small", bufs=1) as small,
            tc.tile_pool(name="scp", bufs=3) as scp,
            tc.tile_pool(name="d32p", bufs=2) as d32p,
            tc.tile_pool(name="hbuf", bufs=6) as hbuf,
            tc.tile_pool(name="psz", bufs=3, space="PSUM") as pszp,
            tc.tile_pool(name="psG", bufs=1, space="PSUM") as psGp,
        ):
            # preload the activation-function table while DMAs run
            warm = small.tile([1, 2], f32)
            nc.vector.memset(warm[:], 0.0)
            nc.scalar.activation(out=warm[:], in_=warm[:],
                                 func=mybir.ActivationFunctionType.Copy)

            xm_t = big.tile([P128, XMW], fp8, tag="xmb")
            dt = big.tile([P128, NS], bf16, tag="dt")
            papt_t = big.tile([P128, NT * PCOL], fp8, tag="papt")

            selw_t = small.tile([P128, H], bf16)

            scs = [None] * C
            d32s = [None] * C

            def dma_xm(c0, c1):
                nc.sync.dma_start(out=xm_t[:, RO[c0]:RO[c1]],
                                  in_=xm[:, RO[c0]:RO[c1]])

            def dma_papt(t0, t1):
                p0, p1 = t0 * PCOL, t1 * PCOL
                nc.sync.dma_start(out=papt_t[:, p0:p1], in_=papt[:, p0:p1])

            def scan_c(c):
                o = RO[c]
                w = int(JWS[c])
                sc = scp.tile([P128, JWMAX], f32, tag=f"sc{c % 3}",
                              name=f"scan_{c}")
                nc.vector.tensor_tensor_scan(
                    out=sc[:, 0:w],
                    data0=xm_t[:, o + w:o + 2 * w],
                    data1=xm_t[:, o:o + w],
                    initial=0.0, op0=mybir.AluOpType.mult,
                    op1=mybir.AluOpType.add,
                )
                scs[c] = sc

            def g2_c(c):
                nd = NDCS[c]
                o = RO[c] + 2 * int(JWS[c])
                d32 = d32p.tile([P128, NBMAX], f32, tag=f"d32{c % 2}",
                                name=f"d32_{c}")
                nc.gpsimd.ap_gather(
                    out_ap=d32[:, 0:nd], in_ap=scs[c][:, 0:int(JWS[c])],
                    idxs_ap=xm_t[:, o:o + nd // 8].bitcast(i16),
                    channels=P128, num_elems=int(JWS[c]), d=1, num_idxs=nd,
                )
                d32s[c] = d32

            def cvt_c(c):
                d0, nd = DOFF[c], NDCS[c]
                if c in (3, 4):
                    nc.scalar.activation(
                        out=dt[0:AUGR, d0:d0 + nd],
                        in_=d32s[c][0:AUGR, 0:nd],
                        func=mybir.ActivationFunctionType.Copy,
                    )
                else:
                    nc.vector.tensor_copy(out=dt[0:AUGR, d0:d0 + nd],
                                          in_=d32s[c][0:AUGR, 0:nd])

            # ---------- issue order ----------
            dma_xm(0, 1)
            dma_xm(1, 2)
            dma_xm(2, 3)
            nc.sync.dma_start(out=selw_t[:], in_=selw[:])
            nc.sync.dma_start(out=dt[AUGR:AUGR + 7, :], in_=aug[:])
            dma_xm(3, 4)
            dma_xm(4, 5)
            dma_xm(5, 6)
            dma_xm(6, 7)
            dma_papt(0, 46)
            dma_xm(7, 8)
            dma_xm(8, 9)
            dma_papt(46, NT)
            dma_xm(9, C)

            # interleaved per-chunk pipeline: DVE scans lead Pool gathers by
            # one chunk; converts trail gathers by one
            scan_c(0)
            scan_c(1)
            g2_c(0)
            for c in range(2, C):
                scan_c(c)
                g2_c(c - 1)
                cvt_c(c - 2)
            g2_c(C - 1)
            cvt_c(C - 2)
            cvt_c(C - 1)

            # ---------- phase B: z -> relu -> G (sw-pipelined batches) ----------
            G_ps = psGp.tile([P128, PCOL], f32, tag="G")
            QB = 8
            batches = []
            for c in range(C):
                t = DOFF[c] // 128
                left = TCH[c]
                while left > 0:
                    sz = min(QB, left)
                    batches.append((t, sz))
                    t += sz
                    left -= sz

            def z_mms(t0, m, ps):
                for u in range(m):
                    n0 = (t0 + u) * P128
                    nc.tensor.matmul(
                        out=ps[:, u * H:(u + 1) * H],
                        lhsT=dt[:, n0:n0 + P128], rhs=selw_t[:],
                        start=True, stop=True,
                    )

            def g_mms(t0, m, h1):
                for u in range(m):
                    t = t0 + u
                    nc.tensor.matmul(
                        out=G_ps[:],
                        lhsT=h1[:, u * H:(u + 1) * H],
                        rhs=papt_t[:, t * PCOL:(t + 1) * PCOL],
                        start=(t == 0), stop=(t == NT - 1),
                    )

            prev = None
            NBAT = len(batches)
            for bi, (t0, m) in enumerate(batches):
                ps = pszp.tile([P128, QB * H], f32, tag="z")
                z_mms(t0, m, ps)
                h1 = hbuf.tile([P128, QB * H], fp8, tag="h1")
                if t0 >= DOFF[8] // 128:
                    # DVE drains its cvts before Act drains relus in the
                    # tail; split late relus across Act and DVE
                    hm = ((m + 1) // 2) * H
                    nc.scalar.activation(
                        out=h1[:, :hm], in_=ps[:, :hm],
                        func=mybir.ActivationFunctionType.Relu,
                    )
                    if hm < m * H:
                        nc.vector.tensor_scalar_max(
                            out=h1[:, hm:m * H], in0=ps[:, hm:m * H],
                            scalar1=0.0,
                        )
                else:
                    nc.scalar.activation(
                        out=h1[:, :m * H], in_=ps[:, :m * H],
                        func=mybir.ActivationFunctionType.Relu,
                    )
                if prev is not None:
                    g_mms(*prev)
                prev = (t0, m, h1)
            g_mms(*prev)

            G_sb = small.tile([P128, PCOL], f32)
            nc.vector.tensor_copy(out=G_sb[:], in_=G_ps[:])
            nc.sync.dma_start(out=gout[:], in_=G_sb[:])

    nc.compile()
    return nc


def _preprocess(x, edge_index, batch_idx):
    """Integer/structure preprocessing -> per-core device inputs."""
    src = np.asarray(edge_index[0], dtype=np.int64)
    dst = np.asarray(edge_index[1], dtype=np.int64)

    deg = (np.bincount(dst, minlength=N) + 1).astype(np.float32)
    dis = (1.0 / np.sqrt(deg)).astype(np.float32)
    sq = np.sqrt(deg).astype(np.float32)
    dis_pad = np.zeros(NPAD, np.float32)
    dis_pad[:N] = dis
    sq_pad = np.zeros(NPAD, np.float32)
    sq_pad[:N] = sq

    bi = np.asarray(batch_idx, dtype=np.int64)
    cnt = np.bincount(bi, minlength=B).astype(np.float32)

    x_np = np.asarray(x, dtype=np.float32)
    x_pad = np.zeros((NPAD, IN), np.float32)
    x_pad[:N] = x_np
    disx = x_pad * dis_pad[:, None]          # [NPAD, 6]

    # ---- pooling matrices (dense PA = P @ A) ----
    loop = np.arange(N, dtype=np.int64)
    src2 = np.concatenate([src, loop])
    dst2 = np.concatenate([dst, loop])
    w = (dis[src2] * dis[dst2]).astype(np.float64)
    flat = bi[dst2] * NPAD + src2
    PA = np.bincount(flat, weights=w, minlength=B * NPAD).reshape(B, NPAD)
    PA = PA.astype(np.float32)
    Pm = np.zeros((B, NPAD), np.float32)
    Pm[bi, np.arange(N)] = 1.0
    papt_full = (np.concatenate([PA, Pm], axis=0) * dis_pad[None, :]).T  # [NPAD,128]

    # graph span per core (for the P columns)
    first_graph = np.zeros(NG, np.int64)
    span = np.zeros(NG, np.int64)
    for k in range(NG):
        lo, hi = k * NS, min((k + 1) * NS, N)
        if lo >= N:
            first_graph[k] = B - 1
            span[k] = 1
            continue
        gset = bi[lo:hi]
        first_graph[k] = gset[0]
        span[k] = gset[-1] - gset[0] + 1
        assert span[k] <= PCOL - B, f"graph span {span[k]} > {PCOL - B}"

    # ---- per (core, chunk) streams, edges round-robin balanced on banks ----
    core = dst // NS
    dst_local = dst - core * NS
    chunk = np.searchsorted(np.asarray(DOFF[1:]), dst_local, side="right")
    # sort by (core, chunk, dst_local); bank = rank within group mod NG
    key0 = (core * C + chunk) * NS + dst_local
    order0 = np.argsort(key0, kind="stable")
    grp = (core * C + chunk)[order0]
    rank = np.arange(E) - np.concatenate(
        [[0], np.cumsum(np.bincount(grp, minlength=NG * C))])[grp]
    bank_e = np.empty(E, np.int64)
    bank_e[order0] = rank % NG

    # final order: (core, chunk, bank, dst_local)
    key = (((core * C + chunk) * NG + bank_e)) * NS + dst_local
    order = np.argsort(key, kind="stable")
    src_s = src[order]
    dstl_s = dst_local[order]

    cell = ((core * C + chunk) * NG + bank_e)[order]
    cellcnt = np.bincount(cell, minlength=NG * C * NG)
    cell_starts = np.zeros(NG * C * NG + 1, np.int64)
    np.cumsum(cellcnt, out=cell_starts[1:])
    cc = cellcnt.reshape(NG, C, NG)

    # per-chunk stream widths (+1 lead 0-slot, pad to 32)
    JWS = []
    for c in range(C):
        m = int(cc[:, c, :].max())
        JWS.append(((m + 1 + 31) // 32) * 32)
    RO = _region_layout(JWS)
    XMW = RO[-1]

    # packed per-chunk regions: [xs bf16 bytes | mask fp8 | bidx i16 | pad]
    f8 = ml_dtypes.float8_e4m3
    xm_all = np.zeros((NG, P128, XMW), f8)
    disx_f8 = disx.astype(f8)
    for k in range(NG):
        for c in range(C):
            w = int(JWS[c])
            nd = NDCS[c]
            b0 = RO[c]
            xs_c = np.zeros((P128, w), f8)
            mk_c = np.zeros((P128, w), f8)
            bx_c = np.zeros((P128, nd // 16), np.int16)
            for g in range(NG):
                ci = (k * C + c) * NG + g
                s0, s1 = cell_starts[ci], cell_starts[ci + 1]
                ncell = s1 - s0
                # pre-gathered feature stream (lead slot 0 stays 0.0)
                xs_c[16 * g:16 * g + 6, 1:1 + ncell] = disx_f8[src_s[s0:s1]].T
                # mask: 0 at each dst segment's first edge, 1 inside
                if ncell > 0:
                    dl = dstl_s[s0:s1]
                    m = np.ones(ncell, f8)
                    m[0] = 0.0
                    m[1:][dl[1:] != dl[:-1]] = 0.0
                    mk_c[16 * g:16 * g + 6, 1:1 + ncell] = m

                # per-node segment end positions (0 for empty -> reads the
                # 0.0 lead slot)
                dloc = dstl_s[s0:s1] - DOFF[c]
                cnts = np.bincount(dloc, minlength=nd)
                ends = np.cumsum(cnts)
                bvals = np.where(cnts > 0, ends, 0).astype(np.int64)
                bx_c[16 * g:16 * (g + 1)] = (
                    bvals.reshape(nd // 16, 16).T.astype(np.int16)
                )
            xm_all[k, :, b0:b0 + w] = xs_c
            xm_all[k, :, b0 + w:b0 + 2 * w] = mk_c
            xm_all[k, :, b0 + 2 * w:b0 + 2 * w + nd // 8] = bx_c.view(f8)

    # aug rows for dt[121:128]: 0-5 dis*x own chunk (self loop), 6 sqrt(deg)
    aug_all = np.zeros((NG, 7, NS), ml_dtypes.bfloat16)
    for k in range(NG):
        n0 = k * NS
        aug_all[k, 0:6] = disx[n0:n0 + NS].T.astype(ml_dtypes.bfloat16)
        aug_all[k, 6] = sq_pad[n0:n0 + NS].astype(ml_dtypes.bfloat16)

    # papt per core: 64 PA cols + local P cols, swizzled to the device
    # tile layout [128, NT*PCOL] so the DMA is a contiguous transfer
    papt_all = np.zeros((NG, P128, NT * PCOL), f8)
    for k in range(NG):
        n0 = k * NS
        pk = np.zeros((NS, PCOL), np.float32)
        pk[:, :B] = papt_full[n0:n0 + NS, :B]
        b0, sp = first_graph[k], span[k]
        pk[:, B:B + sp] = papt_full[n0:n0 + NS, B + b0:B + b0 + sp]
        papt_all[k] = (
            pk.reshape(NT, P128, PCOL).transpose(1, 0, 2)
            .reshape(P128, NT * PCOL).astype(f8)
        )

    return {
        "JW": tuple(JWS),
        "JWS": JWS,
        "xm_all": xm_all,
        "aug_all": aug_all,
        "papt_all": papt_all,
        "first_graph": first_graph,
        "span": span,
        "cnt": cnt,
    }


def _head(G, cnt, inputs):
    f = np.float32
    W2 = np.asarray(inputs["W2"], f)
    b2 = np.asarray(inputs["b2"], f)
    Wg = np.asarray(inputs["Wg"], f)
    bg = np.asarray(inputs["bg"], f)
    Et = np.asarray(inputs["Et"], f)
    Ek = np.asarray(inputs["Ek"], f)
    Ev = np.asarray(inputs["Ev"], f)
    Wp = np.asarray(inputs["Wp"], f)
    bp = np.asarray(inputs["bp"], f)
    Ekid = np.asarray(inputs["Ekid"], f)
    Wc = np.asarray(inputs["Wc"], f)
    bc = np.asarray(inputs["bc"], f)
    Wl = np.asarray(inputs["Wl"], f)
    bl = np.asarray(inputs["bl"], f)
    Wm1 = np.asarray(inputs["Wm1"], f)
    bm1 = np.asarray(inputs["bm1"], f)
    Wm2 = np.asarray(inputs["Wm2"], f)
    bm2 = np.asarray(inputs["bm2"], f)
    st = np.asarray(inputs["sol_type_idx"], np.int64)
    sk = np.asarray(inputs["sol_key_idx"], np.int64)
    sv = np.asarray(inputs["sol_val_idx"], np.int64)
    kid = np.asarray(inputs["kernel_id"], np.int64)
    cond = np.asarray(inputs["cond_vec"], f)
    loc = np.asarray(inputs["local_feats"], f)

    relu = lambda a: np.maximum(a, 0.0).astype(f)

    Ph2 = G[:B] @ W2 + cnt[:, None] * b2[None, :] + G[B:]
    g = (Ph2 / np.maximum(cnt, 1.0)[:, None]) @ Wg + bg

    seq_mean = np.concatenate(
        [Et[st].mean(axis=1), Ek[sk].mean(axis=1), Ev[sv].mean(axis=1)], axis=-1
    ).astype(f)
    p = relu(seq_mean @ Wp + bp)
    kvec = Ekid[kid]
    c = relu(cond @ Wc + bc)
    l = relu(loc @ Wl + bl)
    xf = np.concatenate([g, p, kvec, c, l], axis=1).astype(f)
    return (relu(xf @ Wm1 + bm1) @ Wm2 + bm2).astype(f)


def kernel(**inputs) -> np.ndarray:
    from concourse.bass_utils import run_bass_kernel_spmd

    pre = _preprocess(inputs["x"], inputs["edge_index"], inputs["batch_idx"])
    sig = pre["JW"]
    if sig not in _compiled:
        _compiled[sig] = _build_nc(tuple(pre["JWS"]))
    nc = _compiled[sig]

    W1 = np.asarray(inputs["W1"], np.float32)
    b1 = np.asarray(inputs["b1"], np.float32)
    selw = np.zeros((P128, H), ml_dtypes.bfloat16)
    for g in range(NG):
        selw[16 * g:16 * g + 6] = W1.astype(ml_dtypes.bfloat16)
    selw[AUGR:AUGR + 6] = W1.astype(ml_dtypes.bfloat16)
    selw[AUGR + 6] = b1.astype(ml_dtypes.bfloat16)

    in_maps = []
    for k in range(NG):
        in_maps.append({
            "xm": pre["xm_all"][k],
            "aug": pre["aug_all"][k],
            "selw": selw,
            "papt": pre["papt_all"][k],
        })

    res = run_bass_kernel_spmd(nc, in_maps, core_ids=list(range(NG)))

    Gpa = np.zeros((B, H), np.float64)
    Gp = np.zeros((B, H), np.float64)
    for k, r in enumerate(res.results):
        gt = r["gout"].astype(np.float64)      # [128 f, 80 c]
        Gpa += gt[:, :B].T
        b0, sp = pre["first_graph"][k], pre["span"][k]
        Gp[b0:b0 + sp] += gt[:, B:B + sp].T
    G = np.concatenate([Gpa, Gp], axis=0).astype(np.float32)   # [128, H]

    return _head(G, pre["cnt"], inputs)
